# revision 20
# baseline (speedup 1.0000x reference)
"""MDTA (Restormer multi-dconv-head transposed attention) Trainium2 kernel.

Distribution: data-parallel over batch B=8 across 8 NeuronCores (one image
per core, weights replicated, no collectives).

Per-core pipeline (image = 192ch x 128x128, fp32 in/out):
  1. 1x1 qkv conv     : PE matmul fp32, psum -> fp16 `u` (x-padded 130-wide rows)
  2. depthwise 3x3    : split per channel-chunk between
                          - PE   : 9 accumulated diagonal-matmuls (fp16)
                          - DVE  : 9 scalar_tensor_tensor FMAs (fp16, 2x mode)
                          - POOL : same STT on gpsimd
  3. q,k              : fp16 DMA-transposes (xbar) -> [n,c] layout
     gram S=q@k^T     : PE matmul, psum accumulation over all 16384 pixels
     norms            : STT accum_out (sum of squares along free dim)
  4. v                : fp16, spilled to DRAM scratch re-packed 96-aligned
  5. attn finalize    : scale by 1/max(||q||,eps)/max(||k||,eps)*temp, softmax,
                        PE transpose of block-diag attn
  6. out = attn @ v   : PE matmul fp16
     y = w_proj @ out : PE matmul fp16, fp32 output
"""

import os
import numpy as np

# Hardcoded problem shape (nn_MDTA_74045236183622)
B = 8
C = 192
C3 = 3 * C  # 576
H = W = 128
NPIX = H * W  # 16384
NH, DH = 4, 48
EPS = 1e-12

ROWS = 16                 # output rows per spatial tile
NT = H // ROWS            # 8 tiles
TPX = ROWS * W            # 2048 px per tile
WP = W + 2                # x-padded row width in `u`

# u channel chunks: 5 chunks of <=128 channels, covering q(0:192) k(192:384) v(384:576)
CHUNK_OFF = [0, 128, 256, 384, 512]
CHUNK_W = [128, 128, 128, 128, 64]
# engine assignment for the depthwise conv, per chunk: "pe" | "dve" | "pool"
DW_ENGINE = ["dve", "dve", "dve", "pe", "pe"]

TAPS = [(dy, dx) for dy in (-1, 0, 1) for dx in (-1, 0, 1)]  # row-major 3x3


def build_kernel():
    import concourse.bass as bass
    import concourse.tile as tile
    from concourse import bacc, mybir
    from concourse.masks import make_identity

    f32 = mybir.dt.float32
    f16 = mybir.dt.float16

    nc = bacc.Bacc("TRN2", target_bir_lowering=False, debug=False,
                   enable_asserts=False, num_devices=1)

    x_d = nc.dram_tensor("x", (C, H, W), f32, kind="ExternalInput").ap()
    wqkv_d = nc.dram_tensor("w_qkv", (C3, C), f32, kind="ExternalInput").ap()
    wdw_d = nc.dram_tensor("w_dw", (C3, 1, 3, 3), f32, kind="ExternalInput").ap()
    wproj_d = nc.dram_tensor("w_proj", (C, C), f32, kind="ExternalInput").ap()
    temp_d = nc.dram_tensor("temperature", (NH, 1, 1), f32, kind="ExternalInput").ap()
    out_d = nc.dram_tensor("out", (C, H, W), f32, kind="ExternalOutput").ap()

    with tile.TileContext(nc) as tc:
        _emit(tc, bass, mybir, make_identity, f32, f16,
              x_d, wqkv_d, wdw_d, wproj_d, temp_d, out_d)
    nc.compile()
    return nc


def _emit(tc, bass, mybir, make_identity, f32, f16,
          x_d, wqkv_d, wdw_d, wproj_d, temp_d, out_d):
    from contextlib import ExitStack
    ctx = ExitStack()
    nc = tc.nc
    Alu = mybir.AluOpType
    Act = mybir.ActivationFunctionType

    persist = ctx.enter_context(tc.tile_pool(name="persist", bufs=1))
    xpool = ctx.enter_context(tc.tile_pool(name="xpool", bufs=2))
    stpool = ctx.enter_context(tc.tile_pool(name="stage", bufs=2))
    qkpool = ctx.enter_context(tc.tile_pool(name="qkT", bufs=2))
    dram = ctx.enter_context(tc.tile_pool(name="dram", bufs=1, space="DRAM"))
    psA = ctx.enter_context(tc.tile_pool(name="psA", bufs=2, space="PSUM"))
    psB = ctx.enter_context(tc.tile_pool(name="psB", bufs=2, space="PSUM"))
    psG = ctx.enter_context(tc.tile_pool(name="psG", bufs=1, space="PSUM"))
    vpool = ctx.enter_context(tc.tile_pool(name="vload", bufs=3))
    opool = ctx.enter_context(tc.tile_pool(name="oSt", bufs=3))

    # ---------------- setup: weights into SBUF ----------------
    # identities
    ident128 = persist.tile([128, 128], f32)
    make_identity(nc, ident128)
    ident96h = persist.tile([96, 96], f16)
    make_identity(nc, ident96h)

    # natural (contiguous) weight loads, then on-chip PE transposes
    wq_nat = persist.tile([128, 5, C], f32)
    nc.sync.dma_start(wq_nat[:, 0:4, :],
                      wqkv_d[0:512].rearrange("(ci p) c -> p ci c", p=128))
    nc.sync.dma_start(wq_nat[0:64, 4, :], wqkv_d[512:576])

    # w_qkv^T as lhsT: [c_part, o]; K split 128+64
    wqkvT_a = persist.tile([128, C3], f32)
    wqkvT_b = persist.tile([64, C3], f32)
    for ci in range(5):
        m = CHUNK_W[ci]
        o0 = CHUNK_OFF[ci]
        for kc, kw in ((0, 128), (1, 64)):
            wtp = psA.tile([128, 512], f32, tag="psA", name="wtp")
            nc.tensor.transpose(wtp[0:kw, 0:m],
                                wq_nat[0:m, ci, kc * 128:kc * 128 + kw],
                                ident128[0:m, 0:m])
            dst = wqkvT_a if kc == 0 else wqkvT_b
            nc.scalar.copy(dst[0:kw, o0:o0 + m], wtp[0:kw, 0:m])

    # depthwise weights: wdw_sb[p, t, ci] = w_dw[ci*128+p, 0, t//3, t%3]
    wdw_sb = persist.tile([128, 5, 9], f32)
    wdw_flat = wdw_d.rearrange("o one ky kx -> o (one ky kx)")  # (576, 9)
    with nc.allow_non_contiguous_dma(reason="one-time dw weight load"):
        nc.sync.dma_start(
            wdw_sb[:, 0:4, :],
            wdw_flat[0:512].rearrange("(ci p) t -> p ci t", p=128))
        nc.sync.dma_start(wdw_sb[0:64, 4, :], wdw_flat[512:576])

    # diag(w_dw) blocks for the PE depthwise path, fp16
    diag_sb = persist.tile([128, 9, 5, 128], f16)
    for t in range(9):
        for ci in range(5):
            m = CHUNK_W[ci]
            nc.vector.tensor_scalar_mul(
                diag_sb[0:m, t, ci, 0:m], ident128[0:m, 0:m],
                wdw_sb[0:m, ci, t:t + 1])

    # w_proj^T as lhsT: [c_part(96), kc, o] fp16
    wp_nat = persist.tile([96, 2, C], f32)
    nc.sync.dma_start(wp_nat, wproj_d.rearrange("(ko p) c -> p ko c", p=96))
    wp_h = persist.tile([96, 2, C], f16)
    nc.vector.tensor_copy(out=wp_h, in_=wp_nat)
    wpT = persist.tile([96, 2, C], f16)
    for ko in range(2):
        for kc in range(2):
            wtp2 = psA.tile([96, 96], f16, tag="psA", name="wtp2")
            nc.tensor.transpose(wtp2, wp_h[:, ko, kc * 96:kc * 96 + 96],
                                ident96h)
            nc.scalar.copy(wpT[:, kc, ko * 96:ko * 96 + 96], wtp2)

    # persistent working buffers
    u_t = persist.tile([128, 5, ROWS + 2, WP], f16)       # padded qkv-conv output
    nc.vector.memset(u_t, 0.0)                            # zero pads once
    scratch = persist.tile([128, TPX], f32)               # STT square dump
    np_part = persist.tile([128, 3, NT], f32)             # per-tile sum-of-squares
    v_dram = dram.tile([C, NPIX], f16)                    # v spill, 96-aligned rows

    g_ps = [psG.tile([48, 192], f32, name=f"g_ps{h}") for h in range(NH)]  # gram accumulators

    # ---------------- pass 1: per spatial tile ----------------
    for i in range(NT):
        y0 = i * ROWS

        # x rows y0-1 .. y0+16 -> x_t rows 0..17 ; pad rows zeroed
        x_t = xpool.tile([128, 2, ROWS + 2, W], f32)
        lo = max(y0 - 1, 0)
        hi = min(y0 + ROWS + 1, H)
        ur0 = lo - (y0 - 1)
        if i == 0:
            nc.vector.memset(x_t[:, :, 0:1, :], 0.0)
        if i == NT - 1:
            nc.vector.memset(x_t[:, :, ROWS + 1:ROWS + 2, :], 0.0)
        nc.sync.dma_start(x_t[:, 0, ur0:ur0 + (hi - lo), :], x_d[0:128, lo:hi, :])
        nc.sync.dma_start(x_t[0:64, 1, ur0:ur0 + (hi - lo), :], x_d[128:192, lo:hi, :])

        # ---- 1x1 qkv conv: u rows 0..17 (all 18, pads give zero) ----
        rgroups = [(0, 4), (4, 8), (8, 12), (12, 16), (16, 18)]
        for (r0, r1) in rgroups:
            nrows = r1 - r0
            n = nrows * W
            for ci in range(5):
                m = CHUNK_W[ci]
                o0 = CHUNK_OFF[ci]
                ps = psA.tile([128, 512], f32, tag="psA")
                nc.tensor.matmul(
                    ps[0:m, 0:n], lhsT=wqkvT_a[:, o0:o0 + m],
                    rhs=x_t[:, 0, r0:r1, :], start=True, stop=False)
                nc.tensor.matmul(
                    ps[0:m, 0:n], lhsT=wqkvT_b[:, o0:o0 + m],
                    rhs=x_t[0:64, 1, r0:r1, :], start=False, stop=True)
                nc.scalar.copy(u_t[0:m, ci, r0:r1, 1:W + 1],
                               ps[0:m, 0:n].rearrange("p (r c) -> p r c", c=W))

        # ---- depthwise 3x3 ----
        stage = stpool.tile([128, 5, TPX], f16)
        for ci in range(5):
            m = CHUNK_W[ci]
            eng = DW_ENGINE[ci]
            if eng == "pe":
                for og in range(4):  # out rows og*4..og*4+4
                    oy = og * 4
                    ps = psB.tile([128, 512], f32, tag="psB")
                    for t, (dy, dx) in enumerate(TAPS):
                        nc.tensor.matmul(
                            ps[0:m, :],
                            lhsT=diag_sb[0:m, t, ci, 0:m],
                            rhs=u_t[0:m, ci, oy + dy + 1:oy + dy + 5,
                                    dx + 1:dx + 1 + W],
                            start=(t == 0), stop=(t == 8))
                    nc.scalar.copy(
                        stage[0:m, ci, oy * W:(oy + 4) * W], ps[0:m, :])
            else:
                e = nc.vector if eng == "dve" else nc.gpsimd
                ov = stage[0:m, ci, :].rearrange("p (r c) -> p r c", c=W)
                for t, (dy, dx) in enumerate(TAPS):
                    sh = u_t[0:m, ci, dy + 1:dy + 1 + ROWS, dx + 1:dx + 1 + W]
                    if t == 0:
                        e.tensor_scalar_mul(ov, sh, wdw_sb[0:m, ci, t:t + 1])
                    else:
                        e.scalar_tensor_tensor(
                            ov, sh, wdw_sb[0:m, ci, t:t + 1], ov,
                            op0=Alu.mult, op1=Alu.add)

        # ---- norms partial: sum over tile pixels of q^2 / k^2 ----
        for ci in range(3):
            nc.vector.scalar_tensor_tensor(
                scratch, stage[:, ci, :], 1.0, stage[:, ci, :],
                op0=Alu.mult, op1=Alu.mult,
                accum_out=np_part[:, ci, i:i + 1])

        # ---- v -> DRAM (96-aligned channel rows) ----
        nc.sync.dma_start(v_dram[0:128, i * TPX:(i + 1) * TPX], stage[:, 3, :])
        nc.sync.dma_start(v_dram[128:192, i * TPX:(i + 1) * TPX],
                          stage[0:64, 4, :])

        # ---- q,k transpose (fp16 xbar) + gram accumulation ----
        qkT = qkpool.tile([128, 16, 384], f16)
        for ci in range(3):
            for blk in range(16):
                nc.sync.dma_start_transpose(
                    qkT[:, blk, ci * 128:(ci + 1) * 128],
                    stage[:, ci, blk * 128:(blk + 1) * 128])
        for blk in range(16):
            for h in range(NH):
                nc.tensor.matmul(
                    g_ps[h],
                    lhsT=qkT[:, blk, h * 48:h * 48 + 48],
                    rhs=qkT[:, blk, 192:384],
                    start=(i == 0 and blk == 0),
                    stop=(i == NT - 1 and blk == 15))

    # ---------------- pass 2: finalize attention ----------------
    rn = persist.tile([128, 3], f32)
    nc.vector.tensor_reduce(rn, np_part, axis=mybir.AxisListType.X, op=Alu.add)
    nc.scalar.sqrt(rn, rn)
    nc.vector.tensor_scalar_max(rn, rn, EPS)
    nc.vector.reciprocal(rn, rn)

    nrm_dram = dram.tile([128, 3], f32)
    nc.sync.dma_start(nrm_dram, rn)
    # rnq4[:, h] = 1/max(||q_{h*48+d}||, eps); q channel c -> nrm[c%128, c//128]
    rnq4 = persist.tile([48, 4], f32)
    nc.sync.dma_start(rnq4[:, 0:1], nrm_dram[0:48, 0:1])
    nc.sync.dma_start(rnq4[:, 1:2], nrm_dram[48:96, 0:1])
    nc.sync.dma_start(rnq4[0:32, 2:3], nrm_dram[96:128, 0:1])
    nc.sync.dma_start(rnq4[32:48, 2:3], nrm_dram[0:16, 1:2])
    nc.sync.dma_start(rnq4[:, 3:4], nrm_dram[16:64, 1:2])
    # temperature replicated: tg4[p, h] = temp[h]
    tg4 = persist.tile([48, 4], f32)
    nc.gpsimd.dma_start(
        tg4, bass.AP(tensor=temp_d.tensor, offset=temp_d.offset,
                     ap=[[0, 48], [1, 4]]))
    nc.vector.tensor_mul(rnq4, rnq4, tg4)

    rnk_row = persist.tile([1, 192], f32)
    with nc.allow_non_contiguous_dma(reason="tiny norm vector transpose"):
        nc.sync.dma_start(rnk_row[0:1, 0:64],
                          nrm_dram[64:128, 1:2].rearrange("p o -> o p"))
        nc.sync.dma_start(rnk_row[0:1, 64:192],
                          nrm_dram[0:128, 2:3].rearrange("p o -> o p"))
    ones_row = persist.tile([1, 48], f32)
    nc.vector.memset(ones_row, 1.0)
    rnk_bc = persist.tile([48, 192], f32)
    bc_ps = psA.tile([96, 512], f32, tag="psA", name="bc_ps")
    nc.tensor.matmul(bc_ps[0:48, 0:192], lhsT=ones_row, rhs=rnk_row,
                     start=True, stop=True)
    nc.vector.tensor_copy(out=rnk_bc, in_=bc_ps[0:48, 0:192])

    Sg = persist.tile([48, 2, 384], f32)
    attn_g = [persist.tile([96, 96], f16, name=f"attn_g{g}") for g in range(2)]
    bdT = [persist.tile([96, 96], f16, name=f"bdT{g}") for g in range(2)]
    for h in range(NH):
        g, gh = h // 2, h % 2
        nc.vector.tensor_copy(out=Sg[:, g, gh * 192:gh * 192 + 192], in_=g_ps[h])
    for g in range(2):
        nc.vector.memset(attn_g[g], 0.0)

    mx = persist.tile([48, 1], f32)
    sm = persist.tile([48, 1], f32)
    at16 = persist.tile([48, 48], f16)
    for h in range(NH):
        g, gh = h // 2, h % 2
        nc.vector.scalar_tensor_tensor(
            Sg[:, g, gh * 192:gh * 192 + 192],
            Sg[:, g, gh * 192:gh * 192 + 192],
            rnq4[:, h:h + 1], rnk_bc,
            op0=Alu.mult, op1=Alu.mult)
        blkS = Sg[:, g, gh * 192 + h * 48:gh * 192 + h * 48 + 48]
        nc.vector.tensor_reduce(mx, blkS, axis=mybir.AxisListType.X,
                                op=Alu.max, negate=True)
        nc.scalar.activation(blkS, blkS, Act.Exp, bias=mx, scale=1.0)
        nc.vector.tensor_reduce(sm, blkS, axis=mybir.AxisListType.X, op=Alu.add)
        nc.vector.reciprocal(sm, sm)
        if gh == 0:
            nc.vector.tensor_scalar_mul(attn_g[g][0:48, 0:48], blkS, sm)
        else:
            nc.vector.tensor_scalar_mul(at16, blkS, sm)
            nc.sync.dma_start(attn_g[g][48:96, 48:96], at16)

    for g in range(2):
        trp = psA.tile([96, 96], f16, tag="psA")
        nc.tensor.transpose(trp, attn_g[g], ident96h)
        nc.vector.tensor_copy(out=bdT[g], in_=trp)

    # ---------------- pass 2: out = attn @ v ; y = w_proj @ out ----------------
    for pg in range(NPIX // 512):
        px = pg * 512
        vt = vpool.tile([96, 2, 512], f16)
        nc.sync.dma_start(vt, v_dram.rearrange("(g p) n -> p g n", p=96)[:, :, px:px + 512])
        av = opool.tile([96, 2, 512], f16)
        for g in range(2):
            aps = psA.tile([96, 512], f32, tag="psA")
            nc.tensor.matmul(aps, lhsT=bdT[g], rhs=vt[:, g, :],
                             start=True, stop=True)
            nc.scalar.copy(av[:, g, :], aps)
        y_sb = opool.tile([96, 2, 512], f32, tag="y")
        for mo in range(2):
            yps = psA.tile([96, 512], f32, tag="psA")
            nc.tensor.matmul(yps, lhsT=wpT[:, 0, mo * 96:mo * 96 + 96],
                             rhs=av[:, 0, :], start=True, stop=False)
            nc.tensor.matmul(yps, lhsT=wpT[:, 1, mo * 96:mo * 96 + 96],
                             rhs=av[:, 1, :], start=False, stop=True)
            nc.vector.tensor_copy(out=y_sb[:, mo, :], in_=yps)
        nc.sync.dma_start(
            out_d.rearrange("(mo p) h w -> p mo (h w)", p=96)[:, :, px:px + 512],
            y_sb)

    ctx.close()


_CACHE = {}


def _get_runner():
    if "runner" in _CACHE:
        return _CACHE["runner"]

    import jax
    from jax.sharding import Mesh, PartitionSpec
    from jax.experimental.shard_map import shard_map
    from concourse import mybir
    from concourse import bass2jax

    nc = build_kernel()
    bass2jax.install_neuronx_cc_hook()

    partition_name = (nc.partition_id_tensor.name
                      if nc.partition_id_tensor else None)
    in_names, out_names, out_avals, zero_shapes = [], [], [], []
    for alloc in nc.m.functions[0].allocations:
        if not isinstance(alloc, mybir.MemoryLocationSet):
            continue
        name = alloc.memorylocations[0].name
        if alloc.kind == "ExternalInput":
            if name != partition_name:
                in_names.append(name)
        elif alloc.kind == "ExternalOutput":
            out_names.append(name)
            shape = tuple(alloc.tensor_shape)
            dtype = mybir.dt.np(alloc.dtype)
            out_avals.append(jax.core.ShapedArray(shape, dtype))
            zero_shapes.append((shape, dtype))
    n_params = len(in_names)
    all_names = in_names + out_names
    if partition_name is not None:
        all_names = all_names + [partition_name]

    def _body(*args):
        operands = list(args)
        if partition_name is not None:
            operands.append(bass2jax.partition_id_tensor())
        outs = bass2jax._bass_exec_p.bind(
            *operands,
            out_avals=tuple(out_avals),
            in_names=tuple(all_names),
            out_names=tuple(out_names),
            lowering_input_output_aliases=(),
            sim_require_finite=True,
            sim_require_nnan=True,
            nc=nc,
        )
        return tuple(outs)

    devices = jax.devices()[:B]
    mesh = Mesh(np.asarray(devices), ("core",))
    n_outs = len(out_names)
    sharded = jax.jit(
        shard_map(_body, mesh=mesh,
                  in_specs=(PartitionSpec("core"),) * (n_params + n_outs),
                  out_specs=(PartitionSpec("core"),) * n_outs,
                  check_rep=False),
        donate_argnums=tuple(range(n_params, n_params + n_outs)),
        keep_unused=True,
    )
    runner = (sharded, in_names, out_names, zero_shapes)
    _CACHE["runner"] = runner
    return runner


def _prep_inputs(inputs):
    x = np.ascontiguousarray(np.asarray(inputs["x"], dtype=np.float32))
    per_core = {
        "x": x,  # (B, C, H, W): axis0 is already the shard axis
        "w_qkv": np.tile(np.asarray(inputs["w_qkv"], np.float32)[None], (B, 1, 1)),
        "w_dw": np.tile(np.asarray(inputs["w_dw"], np.float32)[None], (B, 1, 1, 1, 1)),
        "w_proj": np.tile(np.asarray(inputs["w_proj"], np.float32)[None], (B, 1, 1)),
        "temperature": np.tile(np.asarray(inputs["temperature"], np.float32)[None],
                               (B, 1, 1, 1)),
    }
    # concat along axis 0: each core's shard must equal the BIR per-core shape
    return {k: v.reshape((-1,) + v.shape[2:]) for k, v in per_core.items()}


def kernel(**inputs) -> np.ndarray:
    sharded, in_names, out_names, zero_shapes = _get_runner()
    flat = _prep_inputs(inputs)
    args = [flat[name] for name in in_names]
    zeros = [np.zeros((B * s[0],) + tuple(s[1:]), dt) for s, dt in zero_shapes]
    outs = sharded(*args, *zeros)
    y = np.asarray(outs[out_names.index("out")])
    return y.reshape(B, C, H, W).astype(np.float32)


if __name__ == "__main__":
    rng = np.random.default_rng(0)
    demo = {
        "x": rng.standard_normal((B, C, H, W), dtype=np.float32),
        "w_qkv": rng.standard_normal((C3, C), dtype=np.float32) / np.sqrt(C),
        "w_dw": rng.standard_normal((C3, 1, 3, 3), dtype=np.float32) / 3.0,
        "w_proj": rng.standard_normal((C, C), dtype=np.float32) / np.sqrt(C),
        "temperature": np.ones((NH, 1, 1), np.float32),
    }
    out = kernel(**demo)
    print(out.shape, out.dtype, np.abs(out).mean())


# revision 21
# speedup vs baseline: 435.9922x; 435.9922x over previous
"""MDTA (Restormer multi-dconv-head transposed attention) Trainium2 kernel.

Distribution: data-parallel over batch B=8 across 8 NeuronCores (one image
per core, weights replicated, no collectives).

Per-core pipeline (image = 192ch x 128x128, fp32 in/out):
  1. 1x1 qkv conv     : PE matmul fp32, psum -> fp16 `u` (x-padded 130-wide rows)
  2. depthwise 3x3    : split per channel-chunk between
                          - PE   : 9 accumulated diagonal-matmuls (fp16)
                          - DVE  : 9 scalar_tensor_tensor FMAs (fp16, 2x mode)
                          - POOL : same STT on gpsimd
  3. q,k              : fp16 DMA-transposes (xbar) -> [n,c] layout
     gram S=q@k^T     : PE matmul, psum accumulation over all 16384 pixels
     norms            : STT accum_out (sum of squares along free dim)
  4. v                : fp16, spilled to DRAM scratch re-packed 96-aligned
  5. attn finalize    : scale by 1/max(||q||,eps)/max(||k||,eps)*temp, softmax,
                        PE transpose of block-diag attn
  6. out = attn @ v   : PE matmul fp16
     y = w_proj @ out : PE matmul fp16, fp32 output
"""

import os
import numpy as np

# Hardcoded problem shape (nn_MDTA_74045236183622)
B = 8
C = 192
C3 = 3 * C  # 576
H = W = 128
NPIX = H * W  # 16384
NH, DH = 4, 48
EPS = 1e-12

ROWS = 16                 # output rows per spatial tile
NT = H // ROWS            # 8 tiles
TPX = ROWS * W            # 2048 px per tile
WP = W + 2                # x-padded row width in `u`

# u channel chunks: 5 chunks of <=128 channels, covering q(0:192) k(192:384) v(384:576)
CHUNK_OFF = [0, 128, 256, 384, 512]
CHUNK_W = [128, 128, 128, 128, 64]
# engine assignment for the depthwise conv, per chunk: "pe" | "dve" | "pool"
DW_ENGINE = ["dve", "dve", "dve", "pe", "pe"]

TAPS = [(dy, dx) for dy in (-1, 0, 1) for dx in (-1, 0, 1)]  # row-major 3x3


def build_kernel():
    import concourse.bass as bass
    import concourse.tile as tile
    from concourse import bacc, mybir
    from concourse.masks import make_identity

    f32 = mybir.dt.float32
    f16 = mybir.dt.float16

    nc = bacc.Bacc("TRN2", target_bir_lowering=False, debug=False,
                   enable_asserts=False, num_devices=1)

    x_d = nc.dram_tensor("x", (C, H, W), f32, kind="ExternalInput").ap()
    wqkv_d = nc.dram_tensor("w_qkv", (C3, C), f32, kind="ExternalInput").ap()
    wdw_d = nc.dram_tensor("w_dw", (C3, 1, 3, 3), f32, kind="ExternalInput").ap()
    wproj_d = nc.dram_tensor("w_proj", (C, C), f32, kind="ExternalInput").ap()
    temp_d = nc.dram_tensor("temperature", (NH, 1, 1), f32, kind="ExternalInput").ap()
    out_d = nc.dram_tensor("out", (C, H, W), f32, kind="ExternalOutput").ap()

    with tile.TileContext(nc) as tc:
        _emit(tc, bass, mybir, make_identity, f32, f16,
              x_d, wqkv_d, wdw_d, wproj_d, temp_d, out_d)
    nc.compile()
    return nc


def _emit(tc, bass, mybir, make_identity, f32, f16,
          x_d, wqkv_d, wdw_d, wproj_d, temp_d, out_d):
    from contextlib import ExitStack
    ctx = ExitStack()
    nc = tc.nc
    Alu = mybir.AluOpType
    Act = mybir.ActivationFunctionType

    persist = ctx.enter_context(tc.tile_pool(name="persist", bufs=1))
    xpool = ctx.enter_context(tc.tile_pool(name="xpool", bufs=2))
    stpool = ctx.enter_context(tc.tile_pool(name="stage", bufs=2))
    qkpool = ctx.enter_context(tc.tile_pool(name="qkT", bufs=2))
    dram = ctx.enter_context(tc.tile_pool(name="dram", bufs=1, space="DRAM"))
    psA = ctx.enter_context(tc.tile_pool(name="psA", bufs=2, space="PSUM"))
    psB = ctx.enter_context(tc.tile_pool(name="psB", bufs=2, space="PSUM"))
    psG = ctx.enter_context(tc.tile_pool(name="psG", bufs=1, space="PSUM"))
    vpool = ctx.enter_context(tc.tile_pool(name="vload", bufs=3))
    opool = ctx.enter_context(tc.tile_pool(name="oSt", bufs=3))

    # ---------------- setup: weights into SBUF ----------------
    # identities
    ident128 = persist.tile([128, 128], f32)
    make_identity(nc, ident128)
    ident96h = persist.tile([96, 96], f16)
    make_identity(nc, ident96h)

    # natural (contiguous) weight loads, then on-chip PE transposes
    wq_nat = persist.tile([128, 5, C], f32)
    nc.sync.dma_start(wq_nat[:, 0:4, :],
                      wqkv_d[0:512].rearrange("(ci p) c -> p ci c", p=128))
    nc.sync.dma_start(wq_nat[0:64, 4, :], wqkv_d[512:576])

    # w_qkv^T as lhsT: [c_part, o]; K split 128+64
    wqkvT_a = persist.tile([128, C3], f32)
    wqkvT_b = persist.tile([64, C3], f32)
    for ci in range(5):
        m = CHUNK_W[ci]
        o0 = CHUNK_OFF[ci]
        for kc, kw in ((0, 128), (1, 64)):
            wtp = psA.tile([128, 512], f32, tag="psA", name="wtp")
            nc.tensor.transpose(wtp[0:kw, 0:m],
                                wq_nat[0:m, ci, kc * 128:kc * 128 + kw],
                                ident128[0:m, 0:m])
            dst = wqkvT_a if kc == 0 else wqkvT_b
            nc.scalar.copy(dst[0:kw, o0:o0 + m], wtp[0:kw, 0:m])

    # depthwise weights: wdw_sb[p, t, ci] = w_dw[ci*128+p, 0, t//3, t%3]
    wdw_sb = persist.tile([128, 5, 9], f32)
    wdw_flat = wdw_d.rearrange("o one ky kx -> o (one ky kx)")  # (576, 9)
    with nc.allow_non_contiguous_dma(reason="one-time dw weight load"):
        nc.sync.dma_start(
            wdw_sb[:, 0:4, :],
            wdw_flat[0:512].rearrange("(ci p) t -> p ci t", p=128))
        nc.sync.dma_start(wdw_sb[0:64, 4, :], wdw_flat[512:576])

    # diag(w_dw) blocks for the PE depthwise path, fp16
    diag_sb = persist.tile([128, 9, 5, 128], f16)
    for t in range(9):
        for ci in range(5):
            m = CHUNK_W[ci]
            nc.vector.tensor_scalar_mul(
                diag_sb[0:m, t, ci, 0:m], ident128[0:m, 0:m],
                wdw_sb[0:m, ci, t:t + 1])

    # w_proj^T as lhsT: [c_part(96), kc, o] fp16
    wp_nat = persist.tile([96, 2, C], f32)
    nc.sync.dma_start(wp_nat, wproj_d.rearrange("(ko p) c -> p ko c", p=96))
    wp_h = persist.tile([96, 2, C], f16)
    nc.vector.tensor_copy(out=wp_h, in_=wp_nat)
    wpT = persist.tile([96, 2, C], f16)
    for ko in range(2):
        for kc in range(2):
            wtp2 = psA.tile([96, 96], f16, tag="psA", name="wtp2")
            nc.tensor.transpose(wtp2, wp_h[:, ko, kc * 96:kc * 96 + 96],
                                ident96h)
            nc.scalar.copy(wpT[:, kc, ko * 96:ko * 96 + 96], wtp2)

    # persistent working buffers
    u_t = persist.tile([128, 5, ROWS + 2, WP], f16)       # padded qkv-conv output
    nc.vector.memset(u_t, 0.0)                            # zero pads once
    scratch = persist.tile([128, TPX], f32)               # STT square dump
    np_part = persist.tile([128, 3, NT], f32)             # per-tile sum-of-squares
    v_dram = dram.tile([C, NPIX], f16)                    # v spill, 96-aligned rows

    g_ps = [psG.tile([48, 192], f32, name=f"g_ps{h}") for h in range(NH)]  # gram accumulators

    # ---------------- pass 1: per spatial tile ----------------
    for i in range(NT):
        y0 = i * ROWS

        # x rows y0-1 .. y0+16 -> x_t rows 0..17 ; pad rows zeroed
        x_t = xpool.tile([128, 2, ROWS + 2, W], f32)
        lo = max(y0 - 1, 0)
        hi = min(y0 + ROWS + 1, H)
        ur0 = lo - (y0 - 1)
        if i == 0:
            nc.vector.memset(x_t[:, :, 0:1, :], 0.0)
        if i == NT - 1:
            nc.vector.memset(x_t[:, :, ROWS + 1:ROWS + 2, :], 0.0)
        nc.sync.dma_start(x_t[:, 0, ur0:ur0 + (hi - lo), :], x_d[0:128, lo:hi, :])
        nc.sync.dma_start(x_t[0:64, 1, ur0:ur0 + (hi - lo), :], x_d[128:192, lo:hi, :])

        # ---- 1x1 qkv conv: u rows 0..17 (all 18, pads give zero) ----
        rgroups = [(0, 4), (4, 8), (8, 12), (12, 16), (16, 18)]
        for (r0, r1) in rgroups:
            nrows = r1 - r0
            n = nrows * W
            for ci in range(5):
                m = CHUNK_W[ci]
                o0 = CHUNK_OFF[ci]
                ps = psA.tile([128, 512], f32, tag="psA")
                nc.tensor.matmul(
                    ps[0:m, 0:n], lhsT=wqkvT_a[:, o0:o0 + m],
                    rhs=x_t[:, 0, r0:r1, :], start=True, stop=False)
                nc.tensor.matmul(
                    ps[0:m, 0:n], lhsT=wqkvT_b[:, o0:o0 + m],
                    rhs=x_t[0:64, 1, r0:r1, :], start=False, stop=True)
                nc.scalar.copy(u_t[0:m, ci, r0:r1, 1:W + 1],
                               ps[0:m, 0:n].rearrange("p (r c) -> p r c", c=W))

        # ---- depthwise 3x3 ----
        stage = stpool.tile([128, 5, TPX], f16)
        for ci in range(5):
            m = CHUNK_W[ci]
            eng = DW_ENGINE[ci]
            if eng == "pe":
                for og in range(4):  # out rows og*4..og*4+4
                    oy = og * 4
                    ps = psB.tile([128, 512], f32, tag="psB")
                    for t, (dy, dx) in enumerate(TAPS):
                        nc.tensor.matmul(
                            ps[0:m, :],
                            lhsT=diag_sb[0:m, t, ci, 0:m],
                            rhs=u_t[0:m, ci, oy + dy + 1:oy + dy + 5,
                                    dx + 1:dx + 1 + W],
                            start=(t == 0), stop=(t == 8))
                    nc.scalar.copy(
                        stage[0:m, ci, oy * W:(oy + 4) * W], ps[0:m, :])
            else:
                e = nc.vector if eng == "dve" else nc.gpsimd
                ov = stage[0:m, ci, :].rearrange("p (r c) -> p r c", c=W)
                for t, (dy, dx) in enumerate(TAPS):
                    sh = u_t[0:m, ci, dy + 1:dy + 1 + ROWS, dx + 1:dx + 1 + W]
                    if t == 0:
                        e.tensor_scalar_mul(ov, sh, wdw_sb[0:m, ci, t:t + 1])
                    else:
                        e.scalar_tensor_tensor(
                            ov, sh, wdw_sb[0:m, ci, t:t + 1], ov,
                            op0=Alu.mult, op1=Alu.add)

        # ---- norms partial: sum over tile pixels of q^2 / k^2 ----
        for ci in range(3):
            nc.vector.scalar_tensor_tensor(
                scratch, stage[:, ci, :], 1.0, stage[:, ci, :],
                op0=Alu.mult, op1=Alu.mult,
                accum_out=np_part[:, ci, i:i + 1])

        # ---- v -> DRAM (96-aligned channel rows) ----
        nc.sync.dma_start(v_dram[0:128, i * TPX:(i + 1) * TPX], stage[:, 3, :])
        nc.sync.dma_start(v_dram[128:192, i * TPX:(i + 1) * TPX],
                          stage[0:64, 4, :])

        # ---- q,k transpose (fp16 xbar) + gram accumulation ----
        qkT = qkpool.tile([128, 16, 384], f16)
        for ci in range(3):
            for blk in range(16):
                nc.sync.dma_start_transpose(
                    qkT[:, blk, ci * 128:(ci + 1) * 128],
                    stage[:, ci, blk * 128:(blk + 1) * 128])
        for blk in range(16):
            for h in range(NH):
                nc.tensor.matmul(
                    g_ps[h],
                    lhsT=qkT[:, blk, h * 48:h * 48 + 48],
                    rhs=qkT[:, blk, 192:384],
                    start=(i == 0 and blk == 0),
                    stop=(i == NT - 1 and blk == 15))

    # ---------------- pass 2: finalize attention ----------------
    rn = persist.tile([128, 3], f32)
    nc.vector.tensor_reduce(rn, np_part, axis=mybir.AxisListType.X, op=Alu.add)
    nc.scalar.sqrt(rn, rn)
    nc.vector.tensor_scalar_max(rn, rn, EPS)
    nc.vector.reciprocal(rn, rn)

    nrm_dram = dram.tile([128, 3], f32)
    nc.sync.dma_start(nrm_dram, rn)
    # rnq4[:, h] = 1/max(||q_{h*48+d}||, eps); q channel c -> nrm[c%128, c//128]
    rnq4 = persist.tile([48, 4], f32)
    nc.sync.dma_start(rnq4[:, 0:1], nrm_dram[0:48, 0:1])
    nc.sync.dma_start(rnq4[:, 1:2], nrm_dram[48:96, 0:1])
    nc.sync.dma_start(rnq4[0:32, 2:3], nrm_dram[96:128, 0:1])
    nc.sync.dma_start(rnq4[32:48, 2:3], nrm_dram[0:16, 1:2])
    nc.sync.dma_start(rnq4[:, 3:4], nrm_dram[16:64, 1:2])
    # temperature replicated: tg4[p, h] = temp[h]
    tg4 = persist.tile([48, 4], f32)
    nc.gpsimd.dma_start(
        tg4, bass.AP(tensor=temp_d.tensor, offset=temp_d.offset,
                     ap=[[0, 48], [1, 4]]))
    nc.vector.tensor_mul(rnq4, rnq4, tg4)

    rnk_row = persist.tile([1, 192], f32)
    with nc.allow_non_contiguous_dma(reason="tiny norm vector transpose"):
        nc.sync.dma_start(rnk_row[0:1, 0:64],
                          nrm_dram[64:128, 1:2].rearrange("p o -> o p"))
        nc.sync.dma_start(rnk_row[0:1, 64:192],
                          nrm_dram[0:128, 2:3].rearrange("p o -> o p"))
    ones_row = persist.tile([1, 48], f32)
    nc.vector.memset(ones_row, 1.0)
    rnk_bc = persist.tile([48, 192], f32)
    bc_ps = psA.tile([96, 512], f32, tag="psA", name="bc_ps")
    nc.tensor.matmul(bc_ps[0:48, 0:192], lhsT=ones_row, rhs=rnk_row,
                     start=True, stop=True)
    nc.vector.tensor_copy(out=rnk_bc, in_=bc_ps[0:48, 0:192])

    Sg = persist.tile([48, 2, 384], f32)
    attn_g = [persist.tile([96, 96], f16, name=f"attn_g{g}") for g in range(2)]
    bdT = [persist.tile([96, 96], f16, name=f"bdT{g}") for g in range(2)]
    for h in range(NH):
        g, gh = h // 2, h % 2
        nc.vector.tensor_copy(out=Sg[:, g, gh * 192:gh * 192 + 192], in_=g_ps[h])
    for g in range(2):
        nc.vector.memset(attn_g[g], 0.0)

    mx = persist.tile([48, 1], f32)
    sm = persist.tile([48, 1], f32)
    at16 = persist.tile([48, 48], f16)
    for h in range(NH):
        g, gh = h // 2, h % 2
        nc.vector.scalar_tensor_tensor(
            Sg[:, g, gh * 192:gh * 192 + 192],
            Sg[:, g, gh * 192:gh * 192 + 192],
            rnq4[:, h:h + 1], rnk_bc,
            op0=Alu.mult, op1=Alu.mult)
        blkS = Sg[:, g, gh * 192 + h * 48:gh * 192 + h * 48 + 48]
        nc.vector.tensor_reduce(mx, blkS, axis=mybir.AxisListType.X,
                                op=Alu.max, negate=True)
        nc.scalar.activation(blkS, blkS, Act.Exp, bias=mx, scale=1.0)
        nc.vector.tensor_reduce(sm, blkS, axis=mybir.AxisListType.X, op=Alu.add)
        nc.vector.reciprocal(sm, sm)
        if gh == 0:
            nc.vector.tensor_scalar_mul(attn_g[g][0:48, 0:48], blkS, sm)
        else:
            nc.vector.tensor_scalar_mul(at16, blkS, sm)
            nc.sync.dma_start(attn_g[g][48:96, 48:96], at16)

    for g in range(2):
        trp = psA.tile([96, 96], f16, tag="psA")
        nc.tensor.transpose(trp, attn_g[g], ident96h)
        nc.vector.tensor_copy(out=bdT[g], in_=trp)

    # ---------------- pass 2: out = attn @ v ; y = w_proj @ out ----------------
    for pg in range(NPIX // 512):
        px = pg * 512
        vt = vpool.tile([96, 2, 512], f16)
        nc.sync.dma_start(vt, v_dram.rearrange("(g p) n -> p g n", p=96)[:, :, px:px + 512])
        av = opool.tile([96, 2, 512], f16)
        for g in range(2):
            aps = psA.tile([96, 512], f32, tag="psA")
            nc.tensor.matmul(aps, lhsT=bdT[g], rhs=vt[:, g, :],
                             start=True, stop=True)
            nc.scalar.copy(av[:, g, :], aps)
        y_sb = opool.tile([96, 2, 512], f32, tag="y")
        for mo in range(2):
            yps = psA.tile([96, 512], f32, tag="psA")
            nc.tensor.matmul(yps, lhsT=wpT[:, 0, mo * 96:mo * 96 + 96],
                             rhs=av[:, 0, :], start=True, stop=False)
            nc.tensor.matmul(yps, lhsT=wpT[:, 1, mo * 96:mo * 96 + 96],
                             rhs=av[:, 1, :], start=False, stop=True)
            nc.vector.tensor_copy(out=y_sb[:, mo, :], in_=yps)
        nc.sync.dma_start(
            out_d.rearrange("(mo p) h w -> p mo (h w)", p=96)[:, :, px:px + 512],
            y_sb)

    ctx.close()


_CACHE = {}


def _get_runner():
    if "runner" in _CACHE:
        return _CACHE["runner"]

    import jax
    from jax.sharding import Mesh, PartitionSpec
    from jax.experimental.shard_map import shard_map
    from concourse import mybir
    from concourse import bass2jax

    nc = build_kernel()
    bass2jax.install_neuronx_cc_hook()

    partition_name = (nc.partition_id_tensor.name
                      if nc.partition_id_tensor else None)
    in_names, out_names, out_avals, zero_shapes = [], [], [], []
    for alloc in nc.m.functions[0].allocations:
        if not isinstance(alloc, mybir.MemoryLocationSet):
            continue
        name = alloc.memorylocations[0].name
        if alloc.kind == "ExternalInput":
            if name != partition_name:
                in_names.append(name)
        elif alloc.kind == "ExternalOutput":
            out_names.append(name)
            shape = tuple(alloc.tensor_shape)
            dtype = mybir.dt.np(alloc.dtype)
            out_avals.append(jax.core.ShapedArray(shape, dtype))
            zero_shapes.append((shape, dtype))
    n_params = len(in_names)
    all_names = in_names + out_names
    if partition_name is not None:
        all_names = all_names + [partition_name]

    def _body(*args):
        operands = list(args)
        if partition_name is not None:
            operands.append(bass2jax.partition_id_tensor())
        outs = bass2jax._bass_exec_p.bind(
            *operands,
            out_avals=tuple(out_avals),
            in_names=tuple(all_names),
            out_names=tuple(out_names),
            lowering_input_output_aliases=(),
            sim_require_finite=True,
            sim_require_nnan=True,
            nc=nc,
        )
        return tuple(outs)

    devices = jax.devices()[:B]
    mesh = Mesh(np.asarray(devices), ("core",))
    n_outs = len(out_names)
    sharded = jax.jit(
        shard_map(_body, mesh=mesh,
                  in_specs=(PartitionSpec("core"),) * (n_params + n_outs),
                  out_specs=(PartitionSpec("core"),) * n_outs,
                  check_rep=False),
        donate_argnums=tuple(range(n_params, n_params + n_outs)),
        keep_unused=True,
    )
    runner = (sharded, in_names, out_names, zero_shapes, mesh)
    _CACHE["runner"] = runner
    return runner


def _prep_inputs(inputs):
    x = np.ascontiguousarray(np.asarray(inputs["x"], dtype=np.float32))
    per_core = {
        "x": x,  # (B, C, H, W): axis0 is already the shard axis
        "w_qkv": np.tile(np.asarray(inputs["w_qkv"], np.float32)[None], (B, 1, 1)),
        "w_dw": np.tile(np.asarray(inputs["w_dw"], np.float32)[None], (B, 1, 1, 1, 1)),
        "w_proj": np.tile(np.asarray(inputs["w_proj"], np.float32)[None], (B, 1, 1)),
        "temperature": np.tile(np.asarray(inputs["temperature"], np.float32)[None],
                               (B, 1, 1, 1)),
    }
    # concat along axis 0: each core's shard must equal the BIR per-core shape
    return {k: v.reshape((-1,) + v.shape[2:]) for k, v in per_core.items()}


def kernel(**inputs) -> np.ndarray:
    sharded, in_names, out_names, zero_shapes, mesh = _get_runner()
    flat = _prep_inputs(inputs)
    args = [flat[name] for name in in_names]
    zeros = [np.zeros((B * s[0],) + tuple(s[1:]), dt) for s, dt in zero_shapes]
    outs = sharded(*args, *zeros)
    y = np.asarray(outs[out_names.index("out")])
    return y.reshape(B, C, H, W).astype(np.float32)


if __name__ == "__main__":
    rng = np.random.default_rng(0)
    demo = {
        "x": rng.standard_normal((B, C, H, W), dtype=np.float32),
        "w_qkv": rng.standard_normal((C3, C), dtype=np.float32) / np.sqrt(C),
        "w_dw": rng.standard_normal((C3, 1, 3, 3), dtype=np.float32) / 3.0,
        "w_proj": rng.standard_normal((C, C), dtype=np.float32) / np.sqrt(C),
        "temperature": np.ones((NH, 1, 1), np.float32),
    }
    out = kernel(**demo)
    print(out.shape, out.dtype, np.abs(out).mean())
